# revision 30
# baseline (speedup 1.0000x reference)
"""BertParser arc-scorer kernel for Trainium2 (8 NeuronCores, SPMD).

Math: score[e] = W_out @ tanh(Wh @ emb[head_e] + Wd @ emb[dep_e] + b_hidden) + b_out
where W_hidden = [Wh | Wd] acting on the concat of head/dep embeddings.

Strategy (per core, E/8 = 32768 arcs):
  1. Precompute Uh = emb @ Wh.T + b/2 and Ud = emb @ Wd.T + b/2 ([4096, 512]
     fp16) on the tensor engine (PE-transposed inputs, fused load pipeline),
     stored block-by-block to DRAM scratch.
  2. Arcs are host-sorted into NB groups by max(head, dep) block so group k
     only needs the first (k+1) table blocks -> its gathers overlap the
     remaining precompute. Groups are padded to fixed capacities (compile-time
     shapes); padding gathers token 0 and is discarded on the host.
  3. Per arc tile: dma_gather (transpose mode, 512 idxs/call, 2 SWDGE queues)
     both tables into a [d-on-partitions, arcs-on-free] fp16 layout, DVE add,
     ACT tanh, PE matmul against W_out chunks (K=128 x4, M=1) + a K=1
     bias-row matmul for b_out, PSUM -> SBUF -> DRAM out.

Arc dim is sharded across the 8 cores; everything else is replicated.
"""

import numpy as np

import concourse.bass as bass
import concourse.mybir as mybir
import concourse.tile as tile
from concourse import bacc
from concourse.bass import ds, ts
from concourse.bass_utils import run_bass_kernel_spmd
from concourse.masks import make_identity

F32 = mybir.dt.float32
FP16 = mybir.dt.float16
I16 = mybir.dt.int16

P = 128
N_TOK = 4096
H = 768
D_HID = 512
N_CORES = 8
E_FULL = 262144
E_CORE = E_FULL // N_CORES

HC = H // P       # 6 contraction chunks
DC = D_HID // P   # 4 hidden-dim chunks
TT = N_TOK // P   # 32 token tiles

GI = 512          # idxs per dma_gather call (HW SWDGE ring limit ~640)
NQ = 2            # SWDGE queues (3-4 corrupt data under load on this HW)
NB = 4            # precompute blocks / arc groups
BLOCK_TOK = N_TOK // NB
DEBUG_FULL_SRC = False      # gathers read full-table APs
DEBUG_NO_INTERLEAVE = False  # emit all blocks before any gather group


def group_caps(e_core: int):
    caps = []
    for k in range(NB):
        exp = e_core * (2 * k + 1) // (NB * NB)
        caps.append(-(-(exp + 512) // 512) * 512)
    return caps


def _tile_sizes(cap: int):
    """Split a group capacity into DVE/ACT-friendly tiles (1024 then 512)."""
    out = []
    while cap >= 1024:
        out.append(1024)
        cap -= 1024
    while cap:
        out.append(512)
        cap -= 512
    return out


def build_program(e_core: int = E_CORE):
    caps = group_caps(e_core)
    e_pad = sum(caps)

    nc = bacc.Bacc("TRN2", target_bir_lowering=False, debug=False,
                   num_devices=N_CORES, num_swdge_queues=NQ)

    emb = nc.dram_tensor("emb", [N_TOK, H], F32, kind="ExternalInput").ap()
    w_hidden = nc.dram_tensor("w_hidden", [D_HID, 2 * H], F32,
                              kind="ExternalInput").ap()
    b_hidden = nc.dram_tensor("b_hidden", [D_HID], F32,
                              kind="ExternalInput").ap()
    w_out = nc.dram_tensor("w_out", [1, D_HID], F32, kind="ExternalInput").ap()
    b_out = nc.dram_tensor("b_out", [1], F32, kind="ExternalInput").ap()
    head16 = nc.dram_tensor("head16", [P, e_pad // 16], I16,
                            kind="ExternalInput").ap()
    dep16 = nc.dram_tensor("dep16", [P, e_pad // 16], I16,
                           kind="ExternalInput").ap()
    scores = nc.dram_tensor("scores", [e_pad], F32, kind="ExternalOutput").ap()

    with tile.TileContext(nc) as tc:
        _build(tc, emb, w_hidden, b_hidden, w_out, b_out, head16, dep16,
               scores, e_pad, caps)
    nc.compile()
    return nc


def _build(tc, emb, w_hidden, b_hid, w_out, b_out, head16, dep16, scores,
           e_pad, caps):
    nc = tc.nc
    Tanh = mybir.ActivationFunctionType.Tanh

    with tc.tile_pool(name="dram", bufs=1, space="DRAM") as dram, \
         tc.tile_pool(name="const", bufs=1) as const:

        uh_d = dram.tile([N_TOK, D_HID], FP16)
        ud_d = dram.tile([N_TOK, D_HID], FP16)

        # ---- indices (already host-wrapped in the 16-partition layout) ----
        head_sb = const.tile([P, e_pad // 16], I16)
        nc.sync.dma_start(head_sb[:], head16)
        dep_sb = const.tile([P, e_pad // 16], I16)
        nc.sync.dma_start(dep_sb[:], dep16)

        # ---- small constants ----
        wout32 = const.tile([P, DC], F32)
        with nc.allow_non_contiguous_dma(reason="2KB W_out partition layout"):
            nc.sync.dma_start(wout32[:], w_out[0].rearrange("(c p) -> p c", p=P))
        wout_sb = const.tile([P, DC], FP16)
        nc.vector.tensor_copy(wout_sb[:], wout32[:])

        ones_sb = const.tile([1, P], FP16)
        nc.gpsimd.memset(ones_sb[:], 1.0)

        b32 = const.tile([1, D_HID], F32)
        nc.sync.dma_start(b32[:], b_hid[None, :])
        bhalf_sb = const.tile([1, D_HID], FP16)
        nc.vector.tensor_scalar_mul(bhalf_sb[:], b32[:], 0.5)

        bout32 = const.tile([1, 1], F32)
        nc.sync.dma_start(bout32[:], b_out[None, :])
        bout_row = const.tile([1, D_HID], FP16)
        nc.vector.tensor_copy(bout_row[:], bout32[:].to_broadcast((1, D_HID)))

        ident = const.tile([P, P], F32)
        make_identity(nc, ident[:])

        with tc.tile_pool(name="wconv", bufs=1) as wc, \
             tc.tile_pool(name="wT", bufs=1) as pw, \
             tc.tile_pool(name="embT", bufs=1) as pt, \
             tc.tile_pool(name="psum_tr", bufs=2, space="PSUM") as ptr, \
             tc.tile_pool(name="econv", bufs=3) as ec, \
             tc.tile_pool(name="psum_pre", bufs=2, space="PSUM") as pp, \
             tc.tile_pool(name="u16", bufs=4) as pu, \
             tc.tile_pool(name="g", bufs=2) as pg, \
             tc.tile_pool(name="sc", bufs=2) as psc, \
             tc.tile_pool(name="psum_red", bufs=2, space="PSUM") as pr:

            # W: load fp32, transpose 128x128 blocks on PE, cast to fp16.
            wT = pw.tile([P, 2 * HC, D_HID], FP16)
            w32 = wc.tile([P, DC, 2 * H], F32)
            nc.sync.dma_start(w32[:], w_hidden.rearrange("(o p) f -> p o f", p=P))
            for c in range(2 * HC):
                for q in range(DC):
                    tps = ptr.tile([P, P], F32, tag="tr")
                    nc.tensor.transpose(tps[:], w32[:, q, ds(c * P, P)], ident[:])
                    nc.any.tensor_copy(wT[:, c, ds(q * P, P)], tps[:])

            embT = pt.tile([P, HC, N_TOK], FP16)
            emb_t = emb.rearrange("(o p) f -> p o f", p=P)
            uh_t = uh_d.rearrange("(o p) f -> p o f", p=P)
            ud_t = ud_d.rearrange("(o p) f -> p o f", p=P)

            def emit_block(b):
                for t in range(b * (TT // NB), (b + 1) * (TT // NB)):
                    e32 = ec.tile([P, H], F32, tag="e32")
                    nc.sync.dma_start(e32[:], emb_t[:, t, :])
                    for c in range(HC):
                        tps = ptr.tile([P, P], F32, tag="tr")
                        nc.tensor.transpose(tps[:], e32[:, ds(c * P, P)],
                                            ident[:])
                        nc.any.tensor_copy(embT[:, c, ds(t * P, P)], tps[:])
                    for base, dst_t in ((0, uh_t), (HC, ud_t)):
                        ps = pp.tile([P, D_HID], F32)
                        for c in range(HC):
                            nc.tensor.matmul(ps[:], lhsT=embT[:, c, ds(t * P, P)],
                                             rhs=wT[:, base + c, :],
                                             start=(c == 0), stop=False)
                        # bias fold: += ones^T @ (b/2)
                        nc.tensor.matmul(ps[:], lhsT=ones_sb[:], rhs=bhalf_sb[:],
                                         start=False, stop=True)
                        u16 = pu.tile([P, D_HID], FP16)
                        nc.any.tensor_copy(u16[:], ps[:])
                        st = nc.sync.dma_start(dst_t[:, t, :], u16[:])
                        store_insts.append(st)

            qn = [0]
            store_insts = []       # table-store DMAs emitted so far
            scores_row = scores.rearrange("(a n) -> a n", a=1)  # [1, e_pad]

            def emit_group(k, a0):
                # Tile's DRAM overlap tracking misses the rearranged-store vs
                # sliced-gather aliasing; add explicit gather->store deps.
                deps = list(store_insts)
                rows = (k + 1) * BLOCK_TOK
                if DEBUG_FULL_SRC:
                    rows = N_TOK
                uh_src = uh_d[0:rows, :]
                ud_src = ud_d[0:rows, :]
                off = a0
                for size in _tile_sizes(caps[k]):
                    G = size // GI
                    gh = pg.tile([P, G, DC, GI], FP16, tag=f"gh{size}")
                    gd = pg.tile([P, G, DC, GI], FP16, tag=f"gd{size}")
                    for g in range(G):
                        col0 = (off + g * GI) // 16
                        ga = nc.gpsimd.dma_gather(
                            gh[:, g], uh_src, head_sb[:, ds(col0, GI // 16)],
                            num_idxs=GI, num_idxs_reg=GI,
                            elem_size=D_HID, transpose=True,
                            queue_num=qn[0] % NQ)
                        qn[0] += 1
                        gb = nc.gpsimd.dma_gather(
                            gd[:, g], ud_src, dep_sb[:, ds(col0, GI // 16)],
                            num_idxs=GI, num_idxs_reg=GI,
                            elem_size=D_HID, transpose=True,
                            queue_num=qn[0] % NQ)
                        qn[0] += 1
                        for st in deps:
                            tile.add_dep_helper(ga.ins, st.ins, sync=True,
                                                reason="gather waits table store")
                            tile.add_dep_helper(gb.ins, st.ins, sync=True,
                                                reason="gather waits table store")
                    ghf = gh[:].rearrange("p g c a -> p (g c a)")
                    gdf = gd[:].rearrange("p g c a -> p (g c a)")
                    nc.vector.tensor_add(ghf, ghf, gdf)
                    nc.scalar.activation(ghf, ghf, Tanh)
                    ps = pr.tile([1, 1024], F32, tag="psr")
                    for g in range(G):
                        pslice = ps[:, ds(g * GI, GI)]
                        for c in range(DC):
                            nc.tensor.matmul(pslice, lhsT=wout_sb[:, ds(c, 1)],
                                             rhs=gh[:, g, c, :],
                                             start=(c == 0), stop=False)
                        nc.tensor.matmul(pslice, lhsT=ones_sb[:, 0:1],
                                         rhs=bout_row[:], start=False, stop=True)
                    sc = psc.tile([1, 1024], F32, tag="sc")
                    nc.any.tensor_copy(sc[:, :size], ps[:, :size])
                    nc.sync.dma_start(scores_row[:, ds(off, size)],
                                      sc[0:1, :size])
                    off += size

            # Interleave: precompute block b, then gather-group b (which only
            # needs table rows < (b+1)*BLOCK_TOK).
            if DEBUG_NO_INTERLEAVE:
                for b in range(NB):
                    emit_block(b)
                a0 = 0
                for b in range(NB):
                    emit_group(b, a0)
                    a0 += caps[b]
            else:
                a0 = 0
                for b in range(NB):
                    emit_block(b)
                    emit_group(b, a0)
                    a0 += caps[b]


_PROGRAM = None


def _get_program():
    global _PROGRAM
    if _PROGRAM is None:
        _PROGRAM = build_program()
    return _PROGRAM


def _wrap_idx(a: np.ndarray) -> np.ndarray:
    # arc j of the core goes to (partition j%16, column j//16), replicated
    # across the 8 gpsimd cores' 16-partition blocks.
    w = np.tile(a.reshape(-1, 16).T, (8, 1))
    return np.ascontiguousarray(w, dtype=np.int16)


def _group_assign(hi: np.ndarray, di: np.ndarray, caps):
    """Sort arcs by max(head, dep) block, pack into fixed-capacity padded
    groups (overflow spills to the next group, which is always legal).
    Returns (head_pad, dep_pad, slot_of_arc)."""
    e = hi.shape[0]
    e_pad = sum(caps)
    blk = np.maximum(hi, di) // BLOCK_TOK
    order = np.argsort(blk, kind="stable")
    head_pad = np.zeros(e_pad, np.int16)
    dep_pad = np.zeros(e_pad, np.int16)
    slot_of_arc = np.empty(e, np.int64)
    counts = np.bincount(blk, minlength=NB)
    start = 0       # index into `order`
    base = 0        # device slot base of current group
    carry = 0       # spilled arcs from previous groups
    for k in range(NB):
        avail = counts[k] + carry
        take = min(avail, caps[k])
        idxs = order[start:start + take]
        head_pad[base:base + take] = hi[idxs]
        dep_pad[base:base + take] = di[idxs]
        slot_of_arc[idxs] = base + np.arange(take)
        start += take
        carry = avail - take
        base += caps[k]
    assert carry == 0, "group spill overflowed the last group"
    return head_pad, dep_pad, slot_of_arc


def make_in_maps(embeddings, W_hidden, b_hidden, W_out, b_out,
                 head_idx, dep_idx):
    emb = np.ascontiguousarray(np.asarray(embeddings), dtype=np.float32)
    wh = np.ascontiguousarray(np.asarray(W_hidden), dtype=np.float32)
    bh = np.ascontiguousarray(np.asarray(b_hidden), dtype=np.float32)
    wo = np.ascontiguousarray(np.asarray(W_out), dtype=np.float32).reshape(1, D_HID)
    bo = np.ascontiguousarray(np.asarray(b_out), dtype=np.float32).reshape(1)
    hi_full = np.asarray(head_idx)
    di_full = np.asarray(dep_idx)
    e_core = hi_full.shape[0] // N_CORES
    caps = group_caps(e_core)
    in_maps, slots = [], []
    for i in range(N_CORES):
        sl = slice(i * e_core, (i + 1) * e_core)
        hi = np.asarray(hi_full[sl], dtype=np.int16)
        di = np.asarray(di_full[sl], dtype=np.int16)
        hp, dp, slot = _group_assign(hi, di, caps)
        slots.append(slot)
        in_maps.append({
            "emb": emb,
            "w_hidden": wh,
            "b_hidden": bh,
            "w_out": wo,
            "b_out": bo,
            "head16": _wrap_idx(hp),
            "dep16": _wrap_idx(dp),
        })
    return in_maps, slots


def run(in_maps, trace: bool = False, **kwargs):
    nc = _get_program()
    return run_bass_kernel_spmd(nc, in_maps, core_ids=list(range(N_CORES)),
                                trace=trace, **kwargs)


def unpack(res, slots):
    outs = []
    for i, slot in enumerate(slots):
        outs.append(res.results[i]["scores"][slot])
    return np.ascontiguousarray(np.concatenate(outs), dtype=np.float32)


def kernel(embeddings, W_hidden, b_hidden, W_out, b_out, head_idx, dep_idx):
    in_maps, slots = make_in_maps(embeddings, W_hidden, b_hidden, W_out, b_out,
                                  head_idx, dep_idx)
    res = run(in_maps, trace=False)
    return unpack(res, slots)
